# revision 1
# baseline (speedup 1.0000x reference)
"""C51 categorical-DQN histogram projection on Trainium2, 8-core data-parallel.

Exact reformulation of the reference scatter-add:
  m[b,j] = sum_a p[b,a] * hat(pos_ba - j),  hat(x) = relu(1 - |x|),
  pos_ba = clip(alpha_b + 0.99*a, 0, 50)  (alpha from reward/mask).
Per row, a 54-wide window of the in-row prefix-sum table P is fetched at a
data-dependent offset (indirect DMA, one offset per partition); window
diffs give atom masses; m = 3 shifted tent multiplies + clip corrections
on bins 0/50. mask=0 rows use a step table (all mass at a virtual atom 25).
"""
import sys
sys.path.insert(0, "/opt/trn_rl_repo")
import numpy as np
from concourse import bass, bacc, mybir, tile
from concourse.bass_utils import run_bass_kernel_spmd

F32 = mybir.dt.float32
I32 = mybir.dt.int32
OP = mybir.AluOpType
AF = mybir.ActivationFunctionType

P = 128
A = 51
B_TOTAL = 1048576
N_CORES = 8
BC = B_TOTAL // N_CORES
GAMMA = 0.99
ASTAR = 25
L, RP = 20, 20
SP = L + 52 + RP
SMIN, SMAX = -(RP - 1), L - 1
G = 32


def _host_consts():
    p = np.arange(P)[:, None]
    g = np.arange(G)[None, :]
    rowbase0 = ((g * P + p) * SP + (L - 1)).astype(np.int32)
    j001n = (-0.01 * np.arange(54, dtype=np.float32))[None, :].repeat(P, 0)
    return rowbase0, j001n


def _build_nc(Bc):
    TILE = P * G
    T = Bc // TILE
    FA = G * A

    nc = bacc.Bacc("TRN2", target_bir_lowering=False, debug=False)
    pr = nc.dram_tensor("pdist", [Bc, A], F32, kind="ExternalInput")
    rr = nc.dram_tensor("reward", [Bc], F32, kind="ExternalInput")
    mm = nc.dram_tensor("mask", [Bc], I32, kind="ExternalInput")
    rowbase_c = nc.dram_tensor("rowbase0", [P, G], I32, kind="ExternalInput")
    j001n_c = nc.dram_tensor("j001n", [P, 54], F32, kind="ExternalInput")
    mo = nc.dram_tensor("mout", [Bc, A], F32, kind="ExternalOutput")
    ptab = nc.dram_tensor("ptab", [Bc * SP, 1], F32, kind="Internal")

    prf = pr[:, :].rearrange("b a -> (b a)")
    mof = mo[:, :].rearrange("b a -> (b a)")
    ptf = ptab[:, :].rearrange("n o -> (n o)")

    def seg(flat, offset, *dims):
        return bass.AP(flat.tensor, offset, list(dims))

    with tile.TileContext(nc) as tc:
        with tc.tile_pool(name="const", bufs=1) as cpool:
            rowbase = cpool.tile([P, G], I32)
            nc.sync.dma_start(out=rowbase[:], in_=rowbase_c[:, :])
            j001n = cpool.tile([P, 54], F32)
            nc.sync.dma_start(out=j001n[:], in_=j001n_c[:, :])
            biases = []
            for k in range(3):
                bk = cpool.tile([P, 1], F32, tag=f"bias{k}")
                nc.vector.memset(bk[:], float(k))
                biases.append(bk)
            bone = cpool.tile([P, 1], F32, tag="bone")
            nc.vector.memset(bone[:], 1.0)
            zz = cpool.tile([P, FA], F32, tag="zz")
            nc.gpsimd.memset(zz[:], 0.0)

            with tc.tile_pool(name="sb", bufs=2) as pool:
                for t in range(T):
                    tbase = t * TILE
                    pt = pool.tile([P, FA], F32, tag="pt")
                    nc.sync.dma_start(
                        out=pt[:], in_=bass.AP(prf.tensor, tbase * A,
                                               [[A, P], [P * A, G], [1, A]]))
                    rt = pool.tile([P, G], F32, tag="rt")
                    nc.sync.dma_start(
                        out=rt[:], in_=bass.AP(rr[:].tensor, tbase, [[1, P], [P, G]]))
                    mkt = pool.tile([P, G], I32, tag="mkt")
                    nc.sync.dma_start(
                        out=mkt[:], in_=bass.AP(mm[:].tensor, tbase, [[1, P], [P, G]]))

                    # phase A: scan + mask=0 step + padded P-table write
                    st = pool.tile([P, 1 + FA], F32, tag="st")
                    nc.vector.memset(st[:, 0:1], 0.0)
                    nc.vector.tensor_tensor_scan(
                        out=st[:, 1:], data0=pt[:], data1=zz[:], initial=0.0,
                        op0=OP.add, op1=OP.add)
                    sth = st[:]

                    def stv(off, *dims):
                        return bass.AP(sth.tensor, sth.offset + off,
                                       [sth.ap[0]] + list(dims))
                    rowstart = stv(0, [A, G])
                    rowend = stv(A, [A, G])

                    notmk = pool.tile([P, G], I32, tag="notmk")
                    nc.vector.tensor_scalar(
                        out=notmk[:], in0=mkt[:], scalar1=1, scalar2=None,
                        op0=OP.bitwise_xor)
                    nc.vector.copy_predicated(
                        out=stv(1, [A, G], [1, ASTAR]),
                        mask=bass.AP(notmk[:].tensor, notmk[:].offset,
                                     [notmk[:].ap[0], [1, G], [0, ASTAR]]),
                        data=stv(0, [A, G], [0, ASTAR]))
                    nc.vector.copy_predicated(
                        out=stv(1 + ASTAR, [A, G], [1, 52 - 1 - ASTAR]),
                        mask=bass.AP(notmk[:].tensor, notmk[:].offset,
                                     [notmk[:].ap[0], [1, G], [0, 52 - 1 - ASTAR]]),
                        data=stv(A, [A, G], [0, 52 - 1 - ASTAR]))

                    nc.sync.dma_start(
                        out=seg(ptf, tbase * SP + L, [SP, P], [P * SP, G], [1, 52]),
                        in_=stv(0, [A, G], [1, 52]))
                    padLt = pool.tile([P, G * L], F32, tag="padLt")
                    nc.vector.tensor_copy(
                        out=bass.AP(padLt[:].tensor, padLt[:].offset,
                                    [padLt[:].ap[0], [L, G], [1, L]]),
                        in_=stv(0, [A, G], [0, L]))
                    nc.sync.dma_start(
                        out=seg(ptf, tbase * SP, [SP, P], [P * SP, G], [1, L]),
                        in_=padLt[:])
                    padRt = pool.tile([P, G * RP], F32, tag="padRt")
                    nc.vector.tensor_copy(
                        out=bass.AP(padRt[:].tensor, padRt[:].offset,
                                    [padRt[:].ap[0], [RP, G], [1, RP]]),
                        in_=stv(A, [A, G], [0, RP]))
                    nc.sync.dma_start(
                        out=seg(ptf, tbase * SP + L + 52, [SP, P], [P * SP, G], [1, RP]),
                        in_=padRt[:])

                    # phase B scalars
                    mf = pool.tile([P, G], F32, tag="mf")
                    nc.vector.tensor_copy(out=mf[:], in_=mkt[:])
                    a1 = pool.tile([P, G], F32, tag="a1")
                    nc.vector.tensor_scalar(out=a1[:], in0=rt[:], scalar1=2.5,
                                            scalar2=0.25, op0=OP.mult, op1=OP.add)
                    qt = pool.tile([P, G], F32, tag="qt")
                    nc.vector.tensor_scalar(out=qt[:], in0=rt[:], scalar1=2.5,
                                            scalar2=25.0, op0=OP.mult, op1=OP.add)
                    nc.vector.tensor_scalar(out=qt[:], in0=qt[:], scalar1=0.0,
                                            scalar2=50.0, op0=OP.max, op1=OP.min)
                    nc.vector.tensor_scalar(out=qt[:], in0=qt[:],
                                            scalar1=GAMMA * ASTAR, scalar2=None,
                                            op0=OP.subtract)
                    al = pool.tile([P, G], F32, tag="al")
                    nc.vector.tensor_tensor(out=al[:], in0=a1[:], in1=qt[:], op=OP.subtract)
                    nc.vector.tensor_tensor(out=al[:], in0=al[:], in1=mf[:], op=OP.mult)
                    nc.vector.tensor_tensor(out=al[:], in0=al[:], in1=qt[:], op=OP.add)
                    sf = pool.tile([P, G], F32, tag="sf")
                    nc.vector.tensor_scalar(out=sf[:], in0=al[:], scalar1=-0.5,
                                            scalar2=None, op0=OP.add)
                    si = pool.tile([P, G], I32, tag="si")
                    nc.vector.tensor_copy(out=si[:], in_=sf[:])
                    nc.vector.tensor_scalar(out=si[:], in0=si[:], scalar1=SMIN,
                                            scalar2=SMAX, op0=OP.max, op1=OP.min)
                    nc.vector.tensor_copy(out=sf[:], in_=si[:])
                    rho = pool.tile([P, G], F32, tag="rho")
                    nc.vector.tensor_scalar(out=rho[:], in0=sf[:], scalar1=-GAMMA,
                                            scalar2=-GAMMA, op0=OP.mult, op1=OP.add)
                    nc.vector.tensor_tensor(out=rho[:], in0=rho[:], in1=al[:], op=OP.add)
                    g0 = pool.tile([P, G], I32, tag="g0")
                    nc.vector.tensor_scalar(out=g0[:], in0=rowbase[:],
                                            scalar1=t * TILE * SP, scalar2=None,
                                            op0=OP.add)
                    nc.vector.tensor_tensor(out=g0[:], in0=g0[:], in1=si[:], op=OP.subtract)

                    W = pool.tile([P, G * 54], F32, tag="W")
                    for g in range(G):
                        nc.gpsimd.indirect_dma_start(
                            out=W[:, g * 54:(g + 1) * 54], out_offset=None,
                            in_=ptab[:, :],
                            in_offset=bass.IndirectOffsetOnAxis(
                                ap=g0[:, g:g + 1], axis=0))
                    Wh = W[:]

                    def wv(off, *dims):
                        return bass.AP(Wh.tensor, Wh.offset + off,
                                       [Wh.ap[0]] + list(dims))
                    wd = pool.tile([P, G * 53], F32, tag="wd")
                    nc.vector.tensor_tensor(
                        out=wd[:], in0=wv(1, [54, G], [1, 53]),
                        in1=wv(0, [54, G], [1, 53]), op=OP.subtract)
                    wdh = wd[:]

                    def wdv(off, *dims):
                        return bass.AP(wdh.tensor, wdh.offset + off,
                                       [wdh.ap[0]] + list(dims))
                    Y = pool.tile([P, G * 54], F32, tag="Y")
                    nc.vector.tensor_tensor(
                        out=Y[:],
                        in0=bass.AP(rho[:].tensor, rho[:].offset,
                                    [rho[:].ap[0], [1, G], [0, 54]]),
                        in1=bass.AP(j001n[:].tensor, j001n[:].offset,
                                    [j001n[:].ap[0], [0, G], [1, 54]]),
                        op=OP.add)
                    Yh = Y[:]

                    def yv(off, *dims):
                        return bass.AP(Yh.tensor, Yh.offset + off,
                                       [Yh.ap[0]] + list(dims))

                    mt_ = pool.tile([P, FA], F32, tag="mt_")
                    au = pool.tile([P, FA], F32, tag="au")
                    tmp = pool.tile([P, FA], F32, tag="tmp")
                    for k in range(3):
                        nc.scalar.activation(
                            out=au[:], in_=yv(k, [54, G], [1, A]),
                            func=AF.Abs, bias=biases[k][:], scale=1.0)
                        nc.scalar.activation(
                            out=au[:], in_=au[:], func=AF.Relu, bias=bone[:], scale=-1.0)
                        if k == 0:
                            nc.vector.tensor_tensor(
                                out=mt_[:], in0=au[:], in1=wdv(0, [53, G], [1, A]),
                                op=OP.mult)
                        else:
                            nc.vector.tensor_tensor(
                                out=tmp[:], in0=au[:], in1=wdv(k, [53, G], [1, A]),
                                op=OP.mult)
                            nc.vector.tensor_tensor(
                                out=mt_[:], in0=mt_[:], in1=tmp[:], op=OP.add)

                    d0 = pool.tile([P, G], F32, tag="d0")
                    nc.vector.tensor_tensor(out=d0[:], in0=wv(0, [54, G]),
                                            in1=rowstart, op=OP.subtract)
                    cx = pool.tile([P, G], F32, tag="cx")
                    t2 = pool.tile([P, G], F32, tag="t2")
                    for i in (0, 1):
                        nc.vector.tensor_scalar(out=cx[:], in0=rho[:], scalar1=-1.0,
                                                scalar2=-GAMMA * i, op0=OP.mult,
                                                op1=OP.add)
                        nc.vector.tensor_scalar(out=cx[:], in0=cx[:], scalar1=0.0,
                                                scalar2=1.0, op0=OP.max, op1=OP.min)
                        nc.vector.tensor_tensor(out=t2[:], in0=cx[:],
                                                in1=wdv(i, [53, G]), op=OP.mult)
                        nc.vector.tensor_tensor(out=d0[:], in0=d0[:], in1=t2[:],
                                                op=OP.add)
                    d5 = pool.tile([P, G], F32, tag="d5")
                    nc.vector.tensor_tensor(out=d5[:], in0=rowend,
                                            in1=wv(53, [54, G]), op=OP.subtract)
                    for i in (50, 51, 52):
                        nc.vector.tensor_scalar(out=cx[:], in0=rho[:],
                                                scalar1=GAMMA * i - 50.0, scalar2=None,
                                                op0=OP.add)
                        nc.vector.tensor_scalar(out=cx[:], in0=cx[:], scalar1=0.0,
                                                scalar2=1.0, op0=OP.max, op1=OP.min)
                        nc.vector.tensor_tensor(out=t2[:], in0=cx[:],
                                                in1=wdv(i, [53, G]), op=OP.mult)
                        nc.vector.tensor_tensor(out=d5[:], in0=d5[:], in1=t2[:],
                                                op=OP.add)
                    mh = mt_[:]
                    nc.vector.tensor_tensor(
                        out=bass.AP(mh.tensor, mh.offset, [mh.ap[0], [A, G]]),
                        in0=bass.AP(mh.tensor, mh.offset, [mh.ap[0], [A, G]]),
                        in1=d0[:], op=OP.add)
                    nc.vector.tensor_tensor(
                        out=bass.AP(mh.tensor, mh.offset + 50, [mh.ap[0], [A, G]]),
                        in0=bass.AP(mh.tensor, mh.offset + 50, [mh.ap[0], [A, G]]),
                        in1=d5[:], op=OP.add)

                    nc.sync.dma_start(
                        out=bass.AP(mof.tensor, tbase * A, [[A, P], [P * A, G], [1, A]]),
                        in_=mt_[:])
    nc.compile()
    return nc


_NC_CACHE = {}


def kernel(batch_reward, max_next_dist, supports, non_final_mask):
    assert max_next_dist.shape == (B_TOTAL, A)
    if "nc" not in _NC_CACHE:
        _NC_CACHE["nc"] = _build_nc(BC)
    nc = _NC_CACHE["nc"]
    rowbase0, j001n = _host_consts()
    in_maps = []
    for c in range(N_CORES):
        s = slice(c * BC, (c + 1) * BC)
        in_maps.append({
            "pdist": np.ascontiguousarray(max_next_dist[s]).astype(np.float32),
            "reward": np.ascontiguousarray(batch_reward[s]).astype(np.float32),
            "mask": np.ascontiguousarray(non_final_mask[s]).astype(np.int32),
            "rowbase0": rowbase0,
            "j001n": j001n,
        })
    res = run_bass_kernel_spmd(nc, in_maps, core_ids=list(range(N_CORES)))
    return np.concatenate([res.results[c]["mout"] for c in range(N_CORES)], axis=0)

